# revision 29
# baseline (speedup 1.0000x reference)
"""MaxMarginLoss kernel for 8x Trainium2 NeuronCores.

loss = mean_b( sum_c relu(margin - cos(x_b, e_tgt(b)) + cos(x_b, e_c)) - margin )

Strategy: shard the C=100000 classes across 8 cores (padded to 8*12544).
Each core computes per-sample partial hinge sums over its class shard;
the host sums the 8 partial vectors and takes the batch mean.

v2 design:
  - class embeddings pre-transposed on the HOST to [D, CSH]: the device
    loads them straight into matmul layout (no 12.8MB DMA-xbar transposes)
  - per-class norms: ones-matmul on TensorE over esq = eT*eT (free
    partition-broadcast of n2), then a single hand-rolled Rsqrt activation
    (reciprocal_sqrt_and_small table set; measured rel err 4e-5) -- no DVE
    reciprocal (iterative, ~8 cyc/elem) anywhere
  - scores matmul in fp8 e4m3 with DoubleRow perf mode (2 contraction
    rows/instr); operands pre-scaled by 16 to dodge fp8 subnormals, psum
    = 256*cos, bias = 256*(margin - t_cos), final result divided by 256
  - relu+sum epilogue in one pass per [128, CT] psum chunk, split between
    ScalarE (activation Relu + accum) and DVE (scalar_tensor_tensor)
  - e-normalization multiply split DVE / GpSimd; norm chain for tile ct+2
    issued during tile ct so it never gates the matmuls
"""

import numpy as np

B = 1024
D = 512
C = 100000
NCORES = 8
CSH = 12544  # per-core classes, padded (98*128)
CT = 1792  # classes per tile (14*128)
NCT = CSH // CT  # 7
NB = B // 128  # 8 batch chunks
ND = D // 128  # 4 contraction chunks
MARGIN = 0.1
EPS = 1e-8
SCL = 16.0  # fp8 pre-scale; scores come out as SCL^2 * cos
SCL2 = SCL * SCL

# epilogue engine split: batch chunks < NB_S on ScalarE, rest on DVE
NB_S = 6
# normalization dc chunks on gpsimd
GP_DC = (1, 2)

_COMPILED = {}


def _build(stage="full"):
    from contextlib import ExitStack

    import concourse.bacc as bacc
    import concourse.tile as tile
    from concourse import mybir

    f32 = mybir.dt.float32
    bf16 = mybir.dt.bfloat16
    fp8 = mybir.dt.float8e4
    fp16 = mybir.dt.float16
    AF = mybir.ActivationFunctionType
    ALU = mybir.AluOpType
    DR = mybir.MatmulPerfMode.DoubleRow

    nc = bacc.Bacc("TRN2", target_bir_lowering=False, debug=False,
                   num_devices=NCORES)

    def act_rsqrt(out, in_, bias_ap, scale):
        """activation(Rsqrt) without the bass accuracy ban (measured 4e-5)."""
        ins = [nc.scalar.lower_ap(in_), nc.scalar.lower_ap(bias_ap),
               mybir.ImmediateValue(dtype=mybir.dt.float32, value=scale),
               mybir.ImmediateValue(dtype=mybir.dt.float32, value=0.0)]
        return nc.scalar.add_instruction(mybir.InstActivation(
            name=nc.get_next_instruction_name(), func=AF.Rsqrt, ins=ins,
            outs=[nc.scalar.lower_ap(out)]))

    x_d = nc.dram_tensor("xT", [D, B], f32, kind="ExternalInput").ap()
    xn_d = nc.dram_tensor("x", [B, D], f32, kind="ExternalInput").ap()
    tn_d = nc.dram_tensor("temb", [B, D], f32, kind="ExternalInput").ap()
    e_d = nc.dram_tensor("eT", [D, CSH], f32, kind="ExternalInput").ap()
    npad_d = nc.dram_tensor("npad", [128, 1], f32, kind="ExternalInput").ap()
    o_d = nc.dram_tensor("partial", [B], f32, kind="ExternalOutput").ap()

    NB_D = NB - NB_S
    CHUNKS = ((0, 512), (512, 512), (1024, 512), (1536, 256))

    with tile.TileContext(nc) as tc, ExitStack() as ctx:
        singles = ctx.enter_context(tc.tile_pool(name="singles", bufs=1))
        et_pool = ctx.enter_context(tc.tile_pool(name="et", bufs=4))
        esq_pool = ctx.enter_context(tc.tile_pool(name="esq", bufs=2))
        eh_pool = ctx.enter_context(tc.tile_pool(name="eh", bufs=3))
        isc_pool = ctx.enter_context(tc.tile_pool(name="isc", bufs=2))
        rl_pool = ctx.enter_context(tc.tile_pool(name="rl", bufs=2))
        rl2_pool = ctx.enter_context(tc.tile_pool(name="rl2", bufs=2))
        psum_pool = ctx.enter_context(
            tc.tile_pool(name="psum", bufs=2, space="PSUM"))

        n_ct = {"setup": 0, "1ct": 1}.get(stage, NCT)

        # prefetch first class tiles before setup compute
        ets = []

        def fetch(ct):
            er = et_pool.tile([128, ND, CT], bf16, tag="et")
            nc.gpsimd.dma_start(
                out=er,
                in_=e_d[:, ct * CT:(ct + 1) * CT].rearrange(
                    "(i p) c -> p i c", p=128))
            ets.append(er)

        # load order tuned for pipeline head: first class tile, then x/t
        if n_ct > 0:
            fetch(0)

        xTb = singles.tile([128, ND, B], bf16)
        xf = singles.tile([128, NB, D], bf16)
        tf = singles.tile([128, NB, D], bf16)
        npad_sb = singles.tile([128, 1], f32)
        nc.gpsimd.dma_start(out=xTb,
                            in_=x_d.rearrange("(i p) b -> p i b", p=128))
        nc.gpsimd.dma_start(out=xf,
                            in_=xn_d.rearrange("(i p) d -> p i d", p=128))
        if n_ct > 1:
            fetch(1)
        nc.gpsimd.dma_start(out=tf,
                            in_=tn_d.rearrange("(i p) d -> p i d", p=128))
        nc.sync.dma_start(out=npad_sb, in_=npad_d)
        if n_ct > 2:
            fetch(2)

        ones128 = singles.tile([128, 128], bf16)
        nc.vector.memset(ones128, 1.0)
        epsb = singles.tile([128, 1], f32)
        nc.vector.memset(epsb, (EPS / SCL) ** 2)
        zeros = singles.tile([128, CT], bf16)
        nc.vector.memset(zeros, 0.0)

        ehs = {}
        esqs = {}

        def build_esq(ct):
            er = ets[ct]
            esq = esq_pool.tile([128, ND, CT], bf16, tag="esq")
            er2d = er.rearrange("p a c -> p (a c)")
            nc.vector.tensor_tensor(esq.rearrange("p a c -> p (a c)"),
                                    er2d, er2d, op=ALU.mult)
            esqs[ct] = esq

        def build_norm(ct, gp_dcs=GP_DC):
            """ones-matmul norm reduce + rsqrt + normalize to fp8."""
            er = ets[ct]
            esq = esqs.pop(ct)
            npsum = psum_pool.tile([128, CT], f32, tag="ps")
            for off, n in CHUNKS:
                for dc in range(ND):
                    nc.tensor.matmul(npsum[:, off:off + n], lhsT=ones128,
                                     rhs=esq[:, dc, off:off + n],
                                     start=(dc == 0), stop=(dc == ND - 1))
            # iscale16 = SCL / ||e||  (rsqrt(n2/SCL2 + eps')); bf16
            isc = isc_pool.tile([128, CT], bf16, tag="isc")
            act_rsqrt(isc, npsum, epsb, 1.0 / SCL2)
            eh = eh_pool.tile([128, ND, CT], fp8, tag="eh")
            for dc in range(ND):
                eng = nc.gpsimd if dc in gp_dcs else nc.vector
                eng.tensor_tensor(eh[:, dc, :], er[:, dc, :], isc, op=ALU.mult)
            ehs[ct] = eh

        # pipeline head: e-norm chain for tile 0 first, then x chain,
        # then tile 1 norms, then the mt chain (needed latest)
        if n_ct > 0:
            build_esq(0)
        # --- x chain (transposed layout; unlocks the scores matmuls) ---
        xsqT = singles.tile([128, ND, B], bf16)
        xTb2d = xTb.rearrange("p a c -> p (a c)")
        nc.vector.tensor_tensor(xsqT.rearrange("p a c -> p (a c)"),
                                xTb2d, xTb2d, op=ALU.mult)
        if n_ct > 1:
            build_esq(1)
        xn2 = psum_pool.tile([128, CT], f32, tag="ps")
        for off in (0, 512):
            for dc in range(ND):
                nc.tensor.matmul(xn2[:, off:off + 512], lhsT=ones128,
                                 rhs=xsqT[:, dc, off:off + 512],
                                 start=(dc == 0), stop=(dc == ND - 1))
        rx16bc = singles.tile([128, B], fp16)  # SCL/||x_b|| broadcast
        act_rsqrt(rx16bc, xn2[:, 0:B], epsb, 1.0 / SCL2)
        xT8 = singles.tile([128, ND, B], fp8)
        for dc in range(ND):
            nc.vector.tensor_tensor(xT8[:, dc, :], xTb[:, dc, :], rx16bc,
                                    op=ALU.mult)

        # mt chain on ScalarE + GpSimd (DVE-free): mt256 = SCL2*(margin - tcos)
        junk = singles.tile([128, D], bf16)
        nx2 = singles.tile([128, NB], f32)
        nt2 = singles.tile([128, NB], f32)
        dot = singles.tile([128, NB], f32)
        prod = singles.tile([128, NB, D], bf16)
        if n_ct > 0:
            build_norm(0, gp_dcs=())
        nc.gpsimd.tensor_tensor(prod, xf, tf, op=ALU.mult)
        for i in range(NB):
            nc.scalar.activation(junk, xf[:, i, :], AF.Square,
                                 accum_out=nx2[:, i:i + 1])
            nc.scalar.activation(junk, tf[:, i, :], AF.Square,
                                 accum_out=nt2[:, i:i + 1])
            nc.scalar.activation(junk, prod[:, i, :], AF.Identity,
                                 bias=epsb[:, 0:1],
                                 accum_out=dot[:, i:i + 1])
        rx = singles.tile([128, NB], f32)
        rt = singles.tile([128, NB], f32)
        act_rsqrt(rx, nx2, epsb, 1.0)
        act_rsqrt(rt, nt2, epsb, 1.0)
        rxt = singles.tile([128, NB], f32)
        nc.gpsimd.tensor_tensor(rxt, rx, rt, op=ALU.mult)
        tcos = singles.tile([128, NB], f32)
        nc.gpsimd.tensor_tensor(tcos, dot, rxt, op=ALU.mult)
        mt256 = singles.tile([128, NB, 1], f32)
        nc.gpsimd.tensor_scalar(mt256.rearrange("p i o -> p (i o)"), tcos,
                                -SCL2, SCL2 * MARGIN, op0=ALU.mult,
                                op1=ALU.add)
        # padded-row correction (unscaled): corr = npad * relu(mt)
        rm = singles.tile([128, NB], f32)
        nc.gpsimd.tensor_scalar(rm, mt256.rearrange("p i o -> p (i o)"),
                                1.0 / SCL2, 0.0, op0=ALU.mult, op1=ALU.max)
        corr = singles.tile([128, NB], f32)
        nc.gpsimd.tensor_scalar(corr, rm, npad_sb[:, 0:1], None, op0=ALU.mult)


        accS = singles.tile([128, NB_S, NCT], f32)
        accD = singles.tile([128, NB_D, NCT], f32)
        if n_ct < NCT:
            nc.vector.memset(accS, 0.0)
            nc.vector.memset(accD, 0.0)

        # ---------------- main loop over class tiles ----------------

        # (norm chains for tiles 0/1 already issued above)
        for ct in range(n_ct):
            if ct + 3 < n_ct:
                fetch(ct + 3)
            if ct + 2 < n_ct:
                build_esq(ct + 2)
            if ct + 1 < n_ct:
                build_norm(ct + 1)

            eh = ehs.pop(ct)
            for b in range(NB):
                ps = psum_pool.tile([128, CT], f32, tag="ps")
                for j in range(2):
                    for off, n in CHUNKS:
                        nc.tensor.matmul(
                            ps[:, off:off + n],
                            lhsT=xT8[:, 2 * j:2 * j + 2,
                                     128 * b:128 * (b + 1)],
                            rhs=eh[:, 2 * j:2 * j + 2, off:off + n],
                            start=(j == 0), stop=(j == 1), perf_mode=DR,
                            skip_group_check=True)
                if b < NB_S:
                    rl = rl_pool.tile([128, CT], bf16, tag="rl")
                    nc.scalar.activation(
                        rl, ps, AF.Relu, bias=mt256[:, b, :], scale=1.0,
                        accum_out=accS[:, b, ct:ct + 1])
                else:
                    rl2 = rl2_pool.tile([128, CT], bf16, tag="rl2")
                    nc.vector.scalar_tensor_tensor(
                        rl2, ps, mt256[:, b, :], zeros,
                        op0=ALU.add, op1=ALU.max,
                        accum_out=accD[:, b - NB_S, ct:ct + 1])

        # ---------------- finalize ----------------
        res256 = singles.tile([128, NB], f32)
        nc.vector.reduce_sum(out=res256[:, 0:NB_S], in_=accS,
                             axis=mybir.AxisListType.X)
        nc.vector.reduce_sum(out=res256[:, NB_S:NB], in_=accD,
                             axis=mybir.AxisListType.X)
        out = singles.tile([128, NB], f32)
        nc.vector.scalar_tensor_tensor(out, res256, 1.0 / SCL2, corr,
                                       op0=ALU.mult, op1=ALU.subtract)
        nc.sync.dma_start(out=o_d.rearrange("(i p) -> p i", p=128), in_=out)

    nc.compile()
    return nc


def get_nc(stage="full"):
    if stage not in _COMPILED:
        _COMPILED[stage] = _build(stage)
    return _COMPILED[stage]


def make_in_maps(inputs, class_embeddings, targets):
    x = np.ascontiguousarray(np.asarray(inputs, dtype=np.float32))
    ce = np.asarray(class_embeddings, dtype=np.float32)
    tg = np.asarray(targets).astype(np.int64)
    xT = np.ascontiguousarray(x.T)
    temb = np.ascontiguousarray(ce[tg])
    in_maps = []
    for k in range(NCORES):
        lo = k * CSH
        hi = min(lo + CSH, C)
        eT = np.zeros((D, CSH), dtype=np.float32)
        eT[:, :hi - lo] = ce[lo:hi].T
        npad = np.full((128, 1), float(CSH - (hi - lo)), dtype=np.float32)
        in_maps.append({"xT": xT, "x": x, "temb": temb, "eT": eT,
                        "npad": npad})
    return in_maps


def combine(results):
    parts = np.stack([r["partial"] for r in results])  # [8, B]
    per_sample = parts.sum(axis=0) - MARGIN
    return np.float32(per_sample.mean())


def run(inputs, class_embeddings, targets, trace=False, stage="full"):
    from concourse.bass_utils import run_bass_kernel_spmd

    nc = get_nc(stage)
    in_maps = make_in_maps(inputs, class_embeddings, targets)
    res = run_bass_kernel_spmd(nc, in_maps, list(range(NCORES)), trace=trace)
    return combine(res.results), res


def kernel(inputs, class_embeddings, targets):
    out, _ = run(inputs, class_embeddings, targets)
    return out


# revision 31
# speedup vs baseline: 1.0104x; 1.0104x over previous
"""MaxMarginLoss kernel for 8x Trainium2 NeuronCores.

loss = mean_b( sum_c relu(margin - cos(x_b, e_tgt(b)) + cos(x_b, e_c)) - margin )

Strategy: shard the C=100000 classes across 8 cores (padded to 8*12544).
Each core computes per-sample partial hinge sums over its class shard;
the host sums the 8 partial vectors and takes the batch mean.

v2 design:
  - class embeddings pre-transposed on the HOST to [D, CSH]: the device
    loads them straight into matmul layout (no 12.8MB DMA-xbar transposes)
  - per-class norms: ones-matmul on TensorE over esq = eT*eT (free
    partition-broadcast of n2), then a single hand-rolled Rsqrt activation
    (reciprocal_sqrt_and_small table set; measured rel err 4e-5) -- no DVE
    reciprocal (iterative, ~8 cyc/elem) anywhere
  - scores matmul in fp8 e4m3 with DoubleRow perf mode (2 contraction
    rows/instr); operands pre-scaled by 16 to dodge fp8 subnormals, psum
    = 256*cos, bias = 256*(margin - t_cos), final result divided by 256
  - relu+sum epilogue in one pass per [128, CT] psum chunk, split between
    ScalarE (activation Relu + accum) and DVE (scalar_tensor_tensor)
  - e-normalization multiply split DVE / GpSimd; norm chain for tile ct+2
    issued during tile ct so it never gates the matmuls
"""

import numpy as np

B = 1024
D = 512
C = 100000
NCORES = 8
CSH = 12544  # per-core classes, padded (98*128)
CT = 1792  # classes per tile (14*128)
NCT = CSH // CT  # 7
NB = B // 128  # 8 batch chunks
ND = D // 128  # 4 contraction chunks
MARGIN = 0.1
EPS = 1e-8
SCL = 16.0  # fp8 pre-scale; scores come out as SCL^2 * cos
SCL2 = SCL * SCL

# epilogue engine split: batch chunks < NB_S on ScalarE, rest on DVE
NB_S = 6
# normalization dc chunks on gpsimd
GP_DC = (1, 2)

_COMPILED = {}


def _build(stage="full"):
    from contextlib import ExitStack

    import concourse.bacc as bacc
    import concourse.tile as tile
    from concourse import mybir

    f32 = mybir.dt.float32
    bf16 = mybir.dt.bfloat16
    fp8 = mybir.dt.float8e4
    fp16 = mybir.dt.float16
    AF = mybir.ActivationFunctionType
    ALU = mybir.AluOpType
    DR = mybir.MatmulPerfMode.DoubleRow

    nc = bacc.Bacc("TRN2", target_bir_lowering=False, debug=False,
                   num_devices=NCORES)

    def act_rsqrt(out, in_, bias_ap, scale):
        """activation(Rsqrt) without the bass accuracy ban (measured 4e-5)."""
        ins = [nc.scalar.lower_ap(in_), nc.scalar.lower_ap(bias_ap),
               mybir.ImmediateValue(dtype=mybir.dt.float32, value=scale),
               mybir.ImmediateValue(dtype=mybir.dt.float32, value=0.0)]
        return nc.scalar.add_instruction(mybir.InstActivation(
            name=nc.get_next_instruction_name(), func=AF.Rsqrt, ins=ins,
            outs=[nc.scalar.lower_ap(out)]))

    x_d = nc.dram_tensor("xT", [D, B], f32, kind="ExternalInput").ap()
    xn_d = nc.dram_tensor("x", [B, D], f32, kind="ExternalInput").ap()
    tn_d = nc.dram_tensor("temb", [B, D], f32, kind="ExternalInput").ap()
    e_d = nc.dram_tensor("eT", [D, CSH], f32, kind="ExternalInput").ap()
    npad_d = nc.dram_tensor("npad", [128, 1], f32, kind="ExternalInput").ap()
    o_d = nc.dram_tensor("partial", [B], f32, kind="ExternalOutput").ap()

    NB_D = NB - NB_S
    CHUNKS = ((0, 512), (512, 512), (1024, 512), (1536, 256))

    with tile.TileContext(nc) as tc, ExitStack() as ctx:
        singles = ctx.enter_context(tc.tile_pool(name="singles", bufs=1))
        et_pool = ctx.enter_context(tc.tile_pool(name="et", bufs=4))
        esq_pool = ctx.enter_context(tc.tile_pool(name="esq", bufs=2))
        eh_pool = ctx.enter_context(tc.tile_pool(name="eh", bufs=3))
        isc_pool = ctx.enter_context(tc.tile_pool(name="isc", bufs=2))
        rl_pool = ctx.enter_context(tc.tile_pool(name="rl", bufs=2))
        rl2_pool = ctx.enter_context(tc.tile_pool(name="rl2", bufs=2))
        psum_pool = ctx.enter_context(
            tc.tile_pool(name="psum", bufs=2, space="PSUM"))

        TW = [512, 1280] + [1792] * 6  # tile widths, sum = CSH
        TOFF = [sum(TW[:i]) for i in range(len(TW))]
        n_ct = {"setup": 0, "1ct": 1}.get(stage, len(TW))

        def chunks_of(w):
            out, off = [], 0
            while off < w:
                n = min(512, w - off)
                out.append((off, n))
                off += n
            return out

        # prefetch first class tiles before setup compute
        ets = []

        def fetch(ct):
            w = TW[ct]
            er = et_pool.tile([128, ND, w], bf16, tag="et")
            nc.gpsimd.dma_start(
                out=er,
                in_=e_d[:, TOFF[ct]:TOFF[ct] + w].rearrange(
                    "(i p) c -> p i c", p=128))
            ets.append(er)

        # load order tuned for pipeline head: first class tile, then x/t
        if n_ct > 0:
            fetch(0)

        xTb = singles.tile([128, ND, B], bf16)
        xf = singles.tile([128, NB, D], bf16)
        tf = singles.tile([128, NB, D], bf16)
        npad_sb = singles.tile([128, 1], f32)
        nc.gpsimd.dma_start(out=xTb,
                            in_=x_d.rearrange("(i p) b -> p i b", p=128))
        nc.gpsimd.dma_start(out=xf,
                            in_=xn_d.rearrange("(i p) d -> p i d", p=128))
        if n_ct > 1:
            fetch(1)
        nc.gpsimd.dma_start(out=tf,
                            in_=tn_d.rearrange("(i p) d -> p i d", p=128))
        nc.sync.dma_start(out=npad_sb, in_=npad_d)
        if n_ct > 2:
            fetch(2)

        ones128 = singles.tile([128, 128], bf16)
        nc.vector.memset(ones128, 1.0)
        epsb = singles.tile([128, 1], f32)
        nc.vector.memset(epsb, (EPS / SCL) ** 2)
        zeros = singles.tile([128, CT], bf16)
        nc.vector.memset(zeros, 0.0)

        ehs = {}
        esqs = {}

        def build_esq(ct):
            er = ets[ct]
            w = TW[ct]
            esq = esq_pool.tile([128, ND, w], bf16, tag="esq")
            er2d = er.rearrange("p a c -> p (a c)")
            nc.vector.tensor_tensor(esq.rearrange("p a c -> p (a c)"),
                                    er2d, er2d, op=ALU.mult)
            esqs[ct] = esq

        def build_norm(ct, gp_dcs=GP_DC):
            """ones-matmul norm reduce + rsqrt + normalize to fp8."""
            er = ets[ct]
            w = TW[ct]
            esq = esqs.pop(ct)
            npsum = psum_pool.tile([128, w], f32, tag="ps")
            for off, n in chunks_of(w):
                for dc in range(ND):
                    nc.tensor.matmul(npsum[:, off:off + n], lhsT=ones128,
                                     rhs=esq[:, dc, off:off + n],
                                     start=(dc == 0), stop=(dc == ND - 1))
            # iscale16 = SCL / ||e||  (rsqrt(n2/SCL2 + eps')); bf16
            isc = isc_pool.tile([128, w], bf16, tag="isc")
            act_rsqrt(isc, npsum, epsb, 1.0 / SCL2)
            eh = eh_pool.tile([128, ND, w], fp8, tag="eh")
            for dc in range(ND):
                eng = nc.gpsimd if dc in gp_dcs else nc.vector
                eng.tensor_tensor(eh[:, dc, :], er[:, dc, :], isc, op=ALU.mult)
            ehs[ct] = eh

        # pipeline head: e-norm chain for tile 0 first, then x chain,
        # then tile 1 norms, then the mt chain (needed latest)
        if n_ct > 0:
            build_esq(0)
        # --- x chain (transposed layout; unlocks the scores matmuls) ---
        xsqT = singles.tile([128, ND, B], bf16)
        xTb2d = xTb.rearrange("p a c -> p (a c)")
        nc.vector.tensor_tensor(xsqT.rearrange("p a c -> p (a c)"),
                                xTb2d, xTb2d, op=ALU.mult)
        if n_ct > 1:
            build_esq(1)
        xn2 = psum_pool.tile([128, CT], f32, tag="ps")
        for off in (0, 512):
            for dc in range(ND):
                nc.tensor.matmul(xn2[:, off:off + 512], lhsT=ones128,
                                 rhs=xsqT[:, dc, off:off + 512],
                                 start=(dc == 0), stop=(dc == ND - 1))
        rx16bc = singles.tile([128, B], fp16)  # SCL/||x_b|| broadcast
        act_rsqrt(rx16bc, xn2[:, 0:B], epsb, 1.0 / SCL2)
        xT8 = singles.tile([128, ND, B], fp8)
        for dc in range(ND):
            nc.vector.tensor_tensor(xT8[:, dc, :], xTb[:, dc, :], rx16bc,
                                    op=ALU.mult)

        # mt chain on ScalarE + GpSimd (DVE-free): mt256 = SCL2*(margin - tcos)
        junk = singles.tile([128, D], bf16)
        nx2 = singles.tile([128, NB], f32)
        nt2 = singles.tile([128, NB], f32)
        dot = singles.tile([128, NB], f32)
        prod = singles.tile([128, NB, D], bf16)
        if n_ct > 0:
            build_norm(0, gp_dcs=())
        nc.gpsimd.tensor_tensor(prod, xf, tf, op=ALU.mult)
        for i in range(NB):
            nc.scalar.activation(junk, xf[:, i, :], AF.Square,
                                 accum_out=nx2[:, i:i + 1])
            nc.scalar.activation(junk, tf[:, i, :], AF.Square,
                                 accum_out=nt2[:, i:i + 1])
            nc.scalar.activation(junk, prod[:, i, :], AF.Identity,
                                 bias=epsb[:, 0:1],
                                 accum_out=dot[:, i:i + 1])
        rx = singles.tile([128, NB], f32)
        rt = singles.tile([128, NB], f32)
        act_rsqrt(rx, nx2, epsb, 1.0)
        act_rsqrt(rt, nt2, epsb, 1.0)
        rxt = singles.tile([128, NB], f32)
        nc.gpsimd.tensor_tensor(rxt, rx, rt, op=ALU.mult)
        tcos = singles.tile([128, NB], f32)
        nc.gpsimd.tensor_tensor(tcos, dot, rxt, op=ALU.mult)
        mt256 = singles.tile([128, NB, 1], f32)
        nc.gpsimd.tensor_scalar(mt256.rearrange("p i o -> p (i o)"), tcos,
                                -SCL2, SCL2 * MARGIN, op0=ALU.mult,
                                op1=ALU.add)
        # padded-row correction (unscaled): corr = npad * relu(mt)
        rm = singles.tile([128, NB], f32)
        nc.gpsimd.tensor_scalar(rm, mt256.rearrange("p i o -> p (i o)"),
                                1.0 / SCL2, 0.0, op0=ALU.mult, op1=ALU.max)
        corr = singles.tile([128, NB], f32)
        nc.gpsimd.tensor_scalar(corr, rm, npad_sb[:, 0:1], None, op0=ALU.mult)


        accS = singles.tile([128, NB_S, len(TW)], f32)
        accD = singles.tile([128, NB_D, len(TW)], f32)
        if n_ct < len(TW):
            nc.vector.memset(accS, 0.0)
            nc.vector.memset(accD, 0.0)

        # ---------------- main loop over class tiles ----------------

        # (norm chains for tiles 0/1 already issued above)
        for ct in range(n_ct):
            if ct + 3 < n_ct:
                fetch(ct + 3)
            if ct + 2 < n_ct:
                build_esq(ct + 2)
            if ct + 1 < n_ct:
                build_norm(ct + 1)

            eh = ehs.pop(ct)
            w = TW[ct]
            for b in range(NB):
                ps = psum_pool.tile([128, w], f32, tag="ps")
                for j in range(2):
                    for off, n in chunks_of(w):
                        nc.tensor.matmul(
                            ps[:, off:off + n],
                            lhsT=xT8[:, 2 * j:2 * j + 2,
                                     128 * b:128 * (b + 1)],
                            rhs=eh[:, 2 * j:2 * j + 2, off:off + n],
                            start=(j == 0), stop=(j == 1), perf_mode=DR,
                            skip_group_check=True)
                if b < NB_S:
                    rl = rl_pool.tile([128, w], bf16, tag="rl")
                    nc.scalar.activation(
                        rl, ps, AF.Relu, bias=mt256[:, b, :], scale=1.0,
                        accum_out=accS[:, b, ct:ct + 1])
                else:
                    rl2 = rl2_pool.tile([128, w], bf16, tag="rl2")
                    nc.vector.scalar_tensor_tensor(
                        rl2, ps, mt256[:, b, :], zeros[:, 0:w],
                        op0=ALU.add, op1=ALU.max,
                        accum_out=accD[:, b - NB_S, ct:ct + 1])

        # ---------------- finalize ----------------
        res256 = singles.tile([128, NB], f32)
        nc.vector.reduce_sum(out=res256[:, 0:NB_S], in_=accS,
                             axis=mybir.AxisListType.X)
        nc.vector.reduce_sum(out=res256[:, NB_S:NB], in_=accD,
                             axis=mybir.AxisListType.X)
        out = singles.tile([128, NB], f32)
        nc.vector.scalar_tensor_tensor(out, res256, 1.0 / SCL2, corr,
                                       op0=ALU.mult, op1=ALU.subtract)
        nc.sync.dma_start(out=o_d.rearrange("(i p) -> p i", p=128), in_=out)

    nc.compile()
    return nc


def get_nc(stage="full"):
    if stage not in _COMPILED:
        _COMPILED[stage] = _build(stage)
    return _COMPILED[stage]


def make_in_maps(inputs, class_embeddings, targets):
    x = np.ascontiguousarray(np.asarray(inputs, dtype=np.float32))
    ce = np.asarray(class_embeddings, dtype=np.float32)
    tg = np.asarray(targets).astype(np.int64)
    xT = np.ascontiguousarray(x.T)
    temb = np.ascontiguousarray(ce[tg])
    in_maps = []
    for k in range(NCORES):
        lo = k * CSH
        hi = min(lo + CSH, C)
        eT = np.zeros((D, CSH), dtype=np.float32)
        eT[:, :hi - lo] = ce[lo:hi].T
        npad = np.full((128, 1), float(CSH - (hi - lo)), dtype=np.float32)
        in_maps.append({"xT": xT, "x": x, "temb": temb, "eT": eT,
                        "npad": npad})
    return in_maps


def combine(results):
    parts = np.stack([r["partial"] for r in results])  # [8, B]
    per_sample = parts.sum(axis=0) - MARGIN
    return np.float32(per_sample.mean())


def run(inputs, class_embeddings, targets, trace=False, stage="full"):
    from concourse.bass_utils import run_bass_kernel_spmd

    nc = get_nc(stage)
    in_maps = make_in_maps(inputs, class_embeddings, targets)
    res = run_bass_kernel_spmd(nc, in_maps, list(range(NCORES)), trace=trace)
    return combine(res.results), res


def kernel(inputs, class_embeddings, targets):
    out, _ = run(inputs, class_embeddings, targets)
    return out


# revision 32
# speedup vs baseline: 1.0404x; 1.0297x over previous
"""MaxMarginLoss kernel for 8x Trainium2 NeuronCores.

loss = mean_b( sum_c relu(margin - cos(x_b, e_tgt(b)) + cos(x_b, e_c)) - margin )

Strategy: shard the C=100000 classes across 8 cores (padded to 8*12544).
Each core computes per-sample partial hinge sums over its class shard;
the host sums the 8 partial vectors and takes the batch mean.

v2 design:
  - class embeddings pre-transposed on the HOST to [D, CSH]: the device
    loads them straight into matmul layout (no 12.8MB DMA-xbar transposes)
  - per-class norms: ones-matmul on TensorE over esq = eT*eT (free
    partition-broadcast of n2), then a single hand-rolled Rsqrt activation
    (reciprocal_sqrt_and_small table set; measured rel err 4e-5) -- no DVE
    reciprocal (iterative, ~8 cyc/elem) anywhere
  - scores matmul in fp8 e4m3 with DoubleRow perf mode (2 contraction
    rows/instr); operands pre-scaled by 16 to dodge fp8 subnormals, psum
    = 256*cos, bias = 256*(margin - t_cos), final result divided by 256
  - relu+sum epilogue in one pass per [128, CT] psum chunk, split between
    ScalarE (activation Relu + accum) and DVE (scalar_tensor_tensor)
  - e-normalization multiply split DVE / GpSimd; norm chain for tile ct+2
    issued during tile ct so it never gates the matmuls
"""

import numpy as np

B = 1024
D = 512
C = 100000
NCORES = 8
CSH = 12544  # per-core classes, padded (98*128)
CT = 1792  # classes per tile (14*128)
NCT = CSH // CT  # 7
NB = B // 128  # 8 batch chunks
ND = D // 128  # 4 contraction chunks
MARGIN = 0.1
EPS = 1e-8
SCL = 16.0  # fp8 pre-scale; scores come out as SCL^2 * cos
SCL2 = SCL * SCL

# epilogue engine split: batch chunks < NB_S on ScalarE, rest on DVE
NB_S = 6
# normalization dc chunks on gpsimd
GP_DC = (1, 2)

_COMPILED = {}


def _build(stage="full"):
    from contextlib import ExitStack

    import concourse.bacc as bacc
    import concourse.tile as tile
    from concourse import mybir

    f32 = mybir.dt.float32
    bf16 = mybir.dt.bfloat16
    fp8 = mybir.dt.float8e4
    fp16 = mybir.dt.float16
    AF = mybir.ActivationFunctionType
    ALU = mybir.AluOpType
    DR = mybir.MatmulPerfMode.DoubleRow

    nc = bacc.Bacc("TRN2", target_bir_lowering=False, debug=False,
                   num_devices=NCORES)

    def act_rsqrt(out, in_, bias_ap, scale):
        """activation(Rsqrt) without the bass accuracy ban (measured 4e-5)."""
        ins = [nc.scalar.lower_ap(in_), nc.scalar.lower_ap(bias_ap),
               mybir.ImmediateValue(dtype=mybir.dt.float32, value=scale),
               mybir.ImmediateValue(dtype=mybir.dt.float32, value=0.0)]
        return nc.scalar.add_instruction(mybir.InstActivation(
            name=nc.get_next_instruction_name(), func=AF.Rsqrt, ins=ins,
            outs=[nc.scalar.lower_ap(out)]))

    x_d = nc.dram_tensor("xT", [D, B], f32, kind="ExternalInput").ap()
    xn_d = nc.dram_tensor("x", [B, D], f32, kind="ExternalInput").ap()
    tn_d = nc.dram_tensor("temb", [B, D], f32, kind="ExternalInput").ap()
    e_d = nc.dram_tensor("eT", [D, CSH], f32, kind="ExternalInput").ap()
    npad_d = nc.dram_tensor("npad", [128, 1], f32, kind="ExternalInput").ap()
    o_d = nc.dram_tensor("partial", [B], f32, kind="ExternalOutput").ap()

    NB_D = NB - NB_S
    CHUNKS = ((0, 512), (512, 512), (1024, 512), (1536, 256))

    with tile.TileContext(nc) as tc, ExitStack() as ctx:
        singles = ctx.enter_context(tc.tile_pool(name="singles", bufs=1))
        et_pool = ctx.enter_context(tc.tile_pool(name="et", bufs=4))
        esq_pool = ctx.enter_context(tc.tile_pool(name="esq", bufs=2))
        eh_pool = ctx.enter_context(tc.tile_pool(name="eh", bufs=3))
        isc_pool = ctx.enter_context(tc.tile_pool(name="isc", bufs=2))
        rl_pool = ctx.enter_context(tc.tile_pool(name="rl", bufs=2))
        rl2_pool = ctx.enter_context(tc.tile_pool(name="rl2", bufs=2))
        psum_pool = ctx.enter_context(
            tc.tile_pool(name="psum", bufs=2, space="PSUM"))

        TW = [512, 1280] + [1792] * 6  # tile widths, sum = CSH
        TOFF = [sum(TW[:i]) for i in range(len(TW))]
        n_ct = {"setup": 0, "1ct": 1}.get(stage, len(TW))

        def chunks_of(w):
            out, off = [], 0
            while off < w:
                n = min(512, w - off)
                out.append((off, n))
                off += n
            return out

        # prefetch first class tiles before setup compute
        ets = []

        def fetch(ct):
            w = TW[ct]
            er = et_pool.tile([128, ND, w], bf16, tag="et")
            nc.gpsimd.dma_start(
                out=er,
                in_=e_d[:, TOFF[ct]:TOFF[ct] + w].rearrange(
                    "(i p) c -> p i c", p=128))
            ets.append(er)

        # load order tuned for pipeline head: first class tile, then x/t
        if n_ct > 0:
            fetch(0)

        xTb = singles.tile([128, ND, B], bf16)
        xf = singles.tile([128, NB, D], bf16)
        tf = singles.tile([128, NB, D], bf16)
        npad_sb = singles.tile([128, 1], f32)
        nc.gpsimd.dma_start(out=xTb,
                            in_=x_d.rearrange("(i p) b -> p i b", p=128))
        nc.gpsimd.dma_start(out=xf,
                            in_=xn_d.rearrange("(i p) d -> p i d", p=128))
        nc.gpsimd.dma_start(out=tf,
                            in_=tn_d.rearrange("(i p) d -> p i d", p=128))
        nc.sync.dma_start(out=npad_sb, in_=npad_d)
        for _c in range(1, min(3, n_ct)):
            fetch(_c)

        ones128 = singles.tile([128, 128], bf16)
        nc.vector.memset(ones128, 1.0)
        epsb = singles.tile([128, 1], f32)
        nc.vector.memset(epsb, (EPS / SCL) ** 2)
        zeros = singles.tile([128, CT], bf16)
        nc.vector.memset(zeros, 0.0)

        ehs = {}
        esqs = {}

        def build_esq(ct):
            er = ets[ct]
            w = TW[ct]
            esq = esq_pool.tile([128, ND, w], bf16, tag="esq")
            er2d = er.rearrange("p a c -> p (a c)")
            nc.vector.tensor_tensor(esq.rearrange("p a c -> p (a c)"),
                                    er2d, er2d, op=ALU.mult)
            esqs[ct] = esq

        def build_norm(ct, gp_dcs=GP_DC):
            """ones-matmul norm reduce + rsqrt + normalize to fp8."""
            er = ets[ct]
            w = TW[ct]
            esq = esqs.pop(ct)
            npsum = psum_pool.tile([128, w], f32, tag="ps")
            for off, n in chunks_of(w):
                for dc in range(ND):
                    nc.tensor.matmul(npsum[:, off:off + n], lhsT=ones128,
                                     rhs=esq[:, dc, off:off + n],
                                     start=(dc == 0), stop=(dc == ND - 1))
            # iscale16 = SCL / ||e||  (rsqrt(n2/SCL2 + eps')); bf16
            isc = isc_pool.tile([128, w], bf16, tag="isc")
            act_rsqrt(isc, npsum, epsb, 1.0 / SCL2)
            eh = eh_pool.tile([128, ND, w], fp8, tag="eh")
            for dc in range(ND):
                eng = nc.gpsimd if dc in gp_dcs else nc.vector
                eng.tensor_tensor(eh[:, dc, :], er[:, dc, :], isc, op=ALU.mult)
            ehs[ct] = eh

        # pipeline head: e-norm chain for tile 0 first, then x chain,
        # then tile 1 norms, then the mt chain (needed latest)
        if n_ct > 0:
            build_esq(0)
        # --- x chain (transposed layout; unlocks the scores matmuls) ---
        xsqT = singles.tile([128, ND, B], bf16)
        xTb2d = xTb.rearrange("p a c -> p (a c)")
        nc.vector.tensor_tensor(xsqT.rearrange("p a c -> p (a c)"),
                                xTb2d, xTb2d, op=ALU.mult)
        if n_ct > 1:
            build_esq(1)
        xn2 = psum_pool.tile([128, CT], f32, tag="ps")
        for off in (0, 512):
            for dc in range(ND):
                nc.tensor.matmul(xn2[:, off:off + 512], lhsT=ones128,
                                 rhs=xsqT[:, dc, off:off + 512],
                                 start=(dc == 0), stop=(dc == ND - 1))
        rx16bc = singles.tile([128, B], fp16)  # SCL/||x_b|| broadcast
        act_rsqrt(rx16bc, xn2[:, 0:B], epsb, 1.0 / SCL2)
        xT8 = singles.tile([128, ND, B], fp8)
        for dc in range(ND):
            nc.vector.tensor_tensor(xT8[:, dc, :], xTb[:, dc, :], rx16bc,
                                    op=ALU.mult)

        # mt chain on ScalarE + GpSimd (DVE-free): mt256 = SCL2*(margin - tcos)
        junk = singles.tile([128, D], bf16)
        nx2 = singles.tile([128, NB], f32)
        nt2 = singles.tile([128, NB], f32)
        dot = singles.tile([128, NB], f32)
        prod = singles.tile([128, NB, D], bf16)
        if n_ct > 0:
            build_norm(0, gp_dcs=())
        for i in range(NB):
            nc.gpsimd.tensor_tensor(prod[:, i, :], xf[:, i, :], tf[:, i, :],
                                    op=ALU.mult)
        for i in range(NB):
            nc.scalar.activation(junk, xf[:, i, :], AF.Square,
                                 accum_out=nx2[:, i:i + 1])
            nc.scalar.activation(junk, tf[:, i, :], AF.Square,
                                 accum_out=nt2[:, i:i + 1])
            nc.scalar.activation(junk, prod[:, i, :], AF.Identity,
                                 bias=epsb[:, 0:1],
                                 accum_out=dot[:, i:i + 1])
        rx = singles.tile([128, NB], f32)
        rt = singles.tile([128, NB], f32)
        act_rsqrt(rx, nx2, epsb, 1.0)
        act_rsqrt(rt, nt2, epsb, 1.0)
        rxt = singles.tile([128, NB], f32)
        nc.gpsimd.tensor_tensor(rxt, rx, rt, op=ALU.mult)
        tcos = singles.tile([128, NB], f32)
        nc.gpsimd.tensor_tensor(tcos, dot, rxt, op=ALU.mult)
        mt256 = singles.tile([128, NB, 1], f32)
        nc.gpsimd.tensor_scalar(mt256.rearrange("p i o -> p (i o)"), tcos,
                                -SCL2, SCL2 * MARGIN, op0=ALU.mult,
                                op1=ALU.add)
        # padded-row correction (unscaled): corr = npad * relu(mt)
        rm = singles.tile([128, NB], f32)
        nc.gpsimd.tensor_scalar(rm, mt256.rearrange("p i o -> p (i o)"),
                                1.0 / SCL2, 0.0, op0=ALU.mult, op1=ALU.max)
        corr = singles.tile([128, NB], f32)
        nc.gpsimd.tensor_scalar(corr, rm, npad_sb[:, 0:1], None, op0=ALU.mult)


        accS = singles.tile([128, NB_S, len(TW)], f32)
        accD = singles.tile([128, NB_D, len(TW)], f32)
        if n_ct < len(TW):
            nc.vector.memset(accS, 0.0)
            nc.vector.memset(accD, 0.0)

        # ---------------- main loop over class tiles ----------------

        # (norm chains for tiles 0/1 already issued above)
        for ct in range(n_ct):
            if ct + 3 < n_ct:
                fetch(ct + 3)
            if ct + 2 < n_ct:
                build_esq(ct + 2)
            if ct + 1 < n_ct:
                build_norm(ct + 1)

            eh = ehs.pop(ct)
            w = TW[ct]
            for b in range(NB):
                ps = psum_pool.tile([128, w], f32, tag="ps")
                for j in range(2):
                    for off, n in chunks_of(w):
                        nc.tensor.matmul(
                            ps[:, off:off + n],
                            lhsT=xT8[:, 2 * j:2 * j + 2,
                                     128 * b:128 * (b + 1)],
                            rhs=eh[:, 2 * j:2 * j + 2, off:off + n],
                            start=(j == 0), stop=(j == 1), perf_mode=DR,
                            skip_group_check=True)
                if b < NB_S:
                    rl = rl_pool.tile([128, w], bf16, tag="rl")
                    nc.scalar.activation(
                        rl, ps, AF.Relu, bias=mt256[:, b, :], scale=1.0,
                        accum_out=accS[:, b, ct:ct + 1])
                else:
                    rl2 = rl2_pool.tile([128, w], bf16, tag="rl2")
                    nc.vector.scalar_tensor_tensor(
                        rl2, ps, mt256[:, b, :], zeros[:, 0:w],
                        op0=ALU.add, op1=ALU.max,
                        accum_out=accD[:, b - NB_S, ct:ct + 1])

        # ---------------- finalize ----------------
        res256 = singles.tile([128, NB], f32)
        nc.vector.reduce_sum(out=res256[:, 0:NB_S], in_=accS,
                             axis=mybir.AxisListType.X)
        nc.vector.reduce_sum(out=res256[:, NB_S:NB], in_=accD,
                             axis=mybir.AxisListType.X)
        out = singles.tile([128, NB], f32)
        nc.vector.scalar_tensor_tensor(out, res256, 1.0 / SCL2, corr,
                                       op0=ALU.mult, op1=ALU.subtract)
        nc.sync.dma_start(out=o_d.rearrange("(i p) -> p i", p=128), in_=out)

    nc.compile()
    return nc


def get_nc(stage="full"):
    if stage not in _COMPILED:
        _COMPILED[stage] = _build(stage)
    return _COMPILED[stage]


def make_in_maps(inputs, class_embeddings, targets):
    x = np.ascontiguousarray(np.asarray(inputs, dtype=np.float32))
    ce = np.asarray(class_embeddings, dtype=np.float32)
    tg = np.asarray(targets).astype(np.int64)
    xT = np.ascontiguousarray(x.T)
    temb = np.ascontiguousarray(ce[tg])
    in_maps = []
    for k in range(NCORES):
        lo = k * CSH
        hi = min(lo + CSH, C)
        eT = np.zeros((D, CSH), dtype=np.float32)
        eT[:, :hi - lo] = ce[lo:hi].T
        npad = np.full((128, 1), float(CSH - (hi - lo)), dtype=np.float32)
        in_maps.append({"xT": xT, "x": x, "temb": temb, "eT": eT,
                        "npad": npad})
    return in_maps


def combine(results):
    parts = np.stack([r["partial"] for r in results])  # [8, B]
    per_sample = parts.sum(axis=0) - MARGIN
    return np.float32(per_sample.mean())


def run(inputs, class_embeddings, targets, trace=False, stage="full"):
    from concourse.bass_utils import run_bass_kernel_spmd

    nc = get_nc(stage)
    in_maps = make_in_maps(inputs, class_embeddings, targets)
    res = run_bass_kernel_spmd(nc, in_maps, list(range(NCORES)), trace=trace)
    return combine(res.results), res


def kernel(inputs, class_embeddings, targets):
    out, _ = run(inputs, class_embeddings, targets)
    return out
